# revision 1
# baseline (speedup 1.0000x reference)
"""Trainium2 Bass kernel for a 16-head decoder self-attention block (v2, fp16).

Reference computation (B=2, S=2048, E=2048, H=16, D=128):
    qkv = X @ W_qkv.T + b_qkv ; RoPE(Q, K) ; attn = softmax(QK^T/sqrt(D) + mask)
    out = (attn @ V reshaped) @ W_o.T + b_o

Sharding over 8 NeuronCores: data parallel over batch (2) x tensor parallel
over 4 head-groups of 4 heads each. Each core computes its group's qkv
projection, attention, and a partial (rank-512) slice of the output
projection; the host sums the 4 partials per batch element.

v2 design vs the f32r baseline:
  - All matmul operands fp16 (same 1 cyc/row PE rate as f32r at >=256 free,
    but 2x faster weight loads via FWL, half the DMA bytes and SBUF).
  - Everything SBUF-resident: no DRAM roundtrip for Q/K/V.
  - V^T -> V transposes on the DMA XBAR (fp16 is 2-byte), not the PE.
  - Softmax denominator off the PE: fp16 pairwise-tree adds on DVE
    accumulate the 16 exp tiles, then one thin ones-matmul per (head, qc)
    does the cross-partition sum (16x fewer PE rows).
  - V bias and output bias folded on the host (softmax rows sum to 1, so
    attn @ (1 b_v^T) = 1 b_v^T; host adds b_o + W_o @ b_v once).
  - fp16 output partials; host sums in fp32.
"""

import math
import sys

import numpy as np

sys.path.insert(0, "/opt/trn_rl_repo")

B, S, E = 2, 2048, 2048
H, D = 16, 128
NCORES = 8
NGROUP = 4          # head groups (tensor parallel)
HPG = H // NGROUP   # heads per group = 4
GE = HPG * D        # group embed width = 512
KT = E // 128       # contraction tiles over E = 16
ST = S // 128       # sequence tiles = 16
SCALE = 1.0 / math.sqrt(D)

_CACHE = {}


def _build():
    """Build + compile the per-core Bass program (same program, all cores)."""
    import concourse.bacc as bacc
    import concourse.mybir as mybir
    import concourse.tile as tile

    F32 = mybir.dt.float32
    F16 = mybir.dt.float16
    EXP = mybir.ActivationFunctionType.Exp
    IDENT = mybir.ActivationFunctionType.Identity

    nc = bacc.Bacc("TRN2", target_bir_lowering=False, debug=False)

    xt = nc.dram_tensor("xt", [E, S], F16, kind="ExternalInput").ap()        # X[b].T
    w1 = nc.dram_tensor("w1", [8 * 128, E], F16, kind="ExternalInput").ap()  # Q/K m-tiles
    wv = nc.dram_tensor("wv", [HPG * 128, E], F16, kind="ExternalInput").ap()
    w2 = nc.dram_tensor("w2", [KT * 128, GE], F16, kind="ExternalInput").ap()
    bq = nc.dram_tensor("bq", [128, 8], F32, kind="ExternalInput").ap()
    mb = nc.dram_tensor("mb", [128, ST], F32, kind="ExternalInput").ap()
    cosx = nc.dram_tensor("cosx", [128, S], F16, kind="ExternalInput").ap()
    sinx = nc.dram_tensor("sinx", [128, S], F32, kind="ExternalInput").ap()
    ones = nc.dram_tensor("ones", [128, 128], F16, kind="ExternalInput").ap()
    perm = nc.dram_tensor("perm", [128, 128], F16, kind="ExternalInput").ap()
    pout = nc.dram_tensor("pout", [E, S], F16, kind="ExternalOutput").ap()

    with tile.TileContext(nc) as tc:
        with tc.tile_pool(name="small", bufs=1) as spool, \
             tc.tile_pool(name="qk", bufs=1) as qkpool, \
             tc.tile_pool(name="vt", bufs=1) as vtpool, \
             tc.tile_pool(name="trig", bufs=1) as trig:
            ones_sb = spool.tile([128, 128], F16, tag="ones")
            nc.sync.dma_start(ones_sb[:], ones)
            mb_sb = spool.tile([128, ST], F32, tag="mb")
            nc.sync.dma_start(mb_sb[:], mb)
            bq_sb = spool.tile([128, 8], F32, tag="bq")
            nc.sync.dma_start(bq_sb[:], bq)
            perm_sb = spool.tile([128, 128], F16, tag="perm")
            nc.sync.dma_start(perm_sb[:], perm)

            # rope'd Q/K tiles (m order Q0,K0,Q1,K1,... so head h is ready
            # after 2(h+1) projections) and V^T tiles, all fp16 SBUF-resident
            qk_sb = [qkpool.tile([128, S], F16, tag=f"qk{m}", name=f"qk{m}")
                     for m in range(8)]
            vt16 = [vtpool.tile([128, S], F16, tag=f"vt{m}", name=f"vt{m}")
                    for m in range(HPG)]

            cos_sb = trig.tile([128, S], F16, tag="cos")
            sin_sb = trig.tile([128, S], F32, tag="sin")

            # ---------------- Phase 1: projections ----------------
            with tc.tile_pool(name="xt", bufs=1) as xpool, \
                 tc.tile_pool(name="w1p", bufs=3) as w1p, \
                 tc.tile_pool(name="qbp", bufs=3) as qbp, \
                 tc.tile_pool(name="rap", bufs=2) as rap, \
                 tc.tile_pool(name="stp", bufs=2) as stp, \
                 tc.tile_pool(name="ps", bufs=2, space="PSUM") as pspool, \
                 tc.tile_pool(name="ps2", bufs=2, space="PSUM") as ps2pool:
                # V weights first: V m-tiles are the first PE work
                wv0 = w1p.tile([128, E], F16, tag="w", name="wv0")
                nc.sync.dma_start(wv0[:], wv[0:128, :])
                xts = []
                for k in range(KT):
                    t = xpool.tile([128, S], F16, tag=f"xt{k}", name=f"xts{k}")
                    # two half-DMAs so the first (m, half=0) matmuls can start
                    # as soon as the first halves land
                    nc.sync.dma_start(t[:, 0:1024],
                                      xt[k * 128:(k + 1) * 128, 0:1024])
                    xts.append(t)
                for k in range(KT):
                    nc.sync.dma_start(xts[k][:, 1024:2048],
                                      xt[k * 128:(k + 1) * 128, 1024:2048])
                wtiles = [("v", 0, wv0)]
                for m in range(1, HPG):
                    t = w1p.tile([128, E], F16, tag="w", name=f"wv{m}")
                    nc.sync.dma_start(t[:], wv[m * 128:(m + 1) * 128, :])
                    wtiles.append(("v", m, t))
                nc.sync.dma_start(cos_sb[:], cosx)
                nc.sync.dma_start(sin_sb[:], sinx)
                for m in range(8):
                    t = w1p.tile([128, E], F16, tag="w", name=f"w1_{m}")
                    nc.sync.dma_start(t[:], w1[m * 128:(m + 1) * 128, :])
                    wtiles.append(("qk", m, t))

                for kind, m, wt in wtiles:
                    for half in range(2):
                        hs = slice(half * 1024, (half + 1) * 1024)
                        ps = pspool.tile([128, 1024], F32, tag="ps")
                        for k in range(KT):
                            for ns in range(2):
                                sl = slice(ns * 512, (ns + 1) * 512)
                                nc.tensor.matmul(
                                    ps[:, sl], wt[:, k * 128:(k + 1) * 128],
                                    xts[k][:, half * 1024 + ns * 512:
                                            half * 1024 + (ns + 1) * 512],
                                    start=(k == 0), stop=(k == KT - 1))
                        if kind == "v":
                            nc.scalar.activation(vt16[m][:, hs], ps[:], IDENT,
                                                 scale=1.0)
                        else:
                            qb = qbp.tile([128, 1024], F16, tag="qb")
                            nc.scalar.activation(qb[:], ps[:], IDENT,
                                                 bias=bq_sb[:, m:m + 1],
                                                 scale=1.0)
                            ps2 = ps2pool.tile([128, 1024], F32, tag="ps2")
                            for ns in range(2):
                                sl = slice(ns * 512, (ns + 1) * 512)
                                nc.tensor.matmul(ps2[:, sl], perm_sb[:],
                                                 qb[:, sl], start=True,
                                                 stop=True)
                            ra = rap.tile([128, 1024], F16, tag="ra")
                            nc.vector.tensor_mul(ra[:], qb[:], cos_sb[:, hs])
                            st = stp.tile([128, 1024], F16, tag="st")
                            nc.vector.tensor_mul(st[:], ps2[:], sin_sb[:, hs])
                            nc.vector.tensor_add(qk_sb[m][:, hs], st[:], ra[:])

            # ---------------- Phase 2: attention per head ----------------
            NQC = 2
            QW = S // NQC    # 1024
            with tc.tile_pool(name="at", bufs=1) as atp, \
                 tc.tile_pool(name="w2p", bufs=1) as w2p:
                at_sb = atp.tile([128, HPG, S], F16, tag="at", name="at_sb")
                w2s = []
                for m in range(KT):
                    t = w2p.tile([128, GE], F16, tag=f"w2_{m}", name=f"w2_{m}")
                    nc.sync.dma_start(t[:], w2[m * 128:(m + 1) * 128, :])
                    w2s.append(t)

                with tc.tile_pool(name="vh", bufs=2) as vhp, \
                     tc.tile_pool(name="ex", bufs=6) as exp_pool, \
                     tc.tile_pool(name="pr", bufs=2) as prp, \
                     tc.tile_pool(name="acc", bufs=2) as accp, \
                     tc.tile_pool(name="rc", bufs=2) as rcp, \
                     tc.tile_pool(name="pss", bufs=2, space="PSUM") as pss_pool, \
                     tc.tile_pool(name="pso", bufs=2, space="PSUM") as pso_pool:
                    for h in range(HPG):
                        qt = qk_sb[2 * h]
                        kt_ = qk_sb[2 * h + 1]
                        # V tiles [sk, d] from V^T via DMA XBAR transpose
                        vh = vhp.tile([128, S], F16, tag="vh", name=f"vh{h}")
                        for ms in range(ST):
                            nc.sync.dma_start_transpose(
                                vh[:, ms * 128:(ms + 1) * 128],
                                vt16[h][:, ms * 128:(ms + 1) * 128])

                        # Interleaved (qc, ms) stream with the AV matmuls
                        # trailing the scores by TRAIL steps: the ACT exp
                        # stream never starves, the PE always has independent
                        # AV work between score-slot waits, and the DVE denom
                        # chain has slack before its ones-matmul.
                        TRAIL = 3
                        total = NQC * ST
                        exs = [None] * total
                        dacc_q = [None, None]
                        acc = None
                        for step in range(total + TRAIL):
                            g = step - TRAIL
                            if g >= 0:
                                pqc, pms = divmod(g, ST)
                                if pms == 0:
                                    cur_pso = pso_pool.tile(
                                        [128, QW], F32, tag="pso",
                                        name=f"pso{h}_{pqc}")
                                for ns in range(2):
                                    sl = slice(ns * 512, (ns + 1) * 512)
                                    nc.tensor.matmul(cur_pso[:, sl],
                                                     vh[:, pms * 128:
                                                        (pms + 1) * 128],
                                                     exs[g][:, sl],
                                                     start=(pms == 0),
                                                     stop=(pms == ST - 1))
                                if pms == ST - 1:
                                    qsl = slice(pqc * QW, (pqc + 1) * QW)
                                    # thin ones-matmul: cross-partition sum of
                                    # the DVE-accumulated exp (borrows a pss
                                    # ring slot briefly)
                                    psd = pss_pool.tile([128, QW], F32,
                                                        tag="pss",
                                                        name=f"psd{h}_{pqc}")
                                    for ns in range(2):
                                        sl = slice(ns * 512, (ns + 1) * 512)
                                        nc.tensor.matmul(
                                            psd[:, sl], ones_sb[:],
                                            dacc_q[pqc][:, sl],
                                            start=True, stop=True)
                                    rc = rcp.tile([128, QW], F32, tag="rc")
                                    nc.vector.reciprocal_approx_fast(rc[:],
                                                                     psd[:])
                                    nc.vector.tensor_mul(at_sb[:, h, qsl],
                                                         cur_pso[:], rc[:])
                            if step < total:
                                qc, ms = divmod(step, ST)
                                pss = pss_pool.tile([128, QW], F32, tag="pss")
                                for ns in range(2):
                                    sl = slice(ns * 512, (ns + 1) * 512)
                                    nc.tensor.matmul(
                                        pss[:, sl],
                                        kt_[:, ms * 128:(ms + 1) * 128],
                                        qt[:, qc * QW + ns * 512:
                                           qc * QW + (ns + 1) * 512],
                                        start=True, stop=True)
                                ex = exp_pool.tile([128, QW], F16, tag="ex")
                                nc.scalar.activation(ex[:], pss[:], EXP,
                                                     bias=mb_sb[:, ms:ms + 1],
                                                     scale=SCALE)
                                exs[step] = ex
                                # fp16 pair + running-chain softmax denom
                                if ms % 2 == 1:
                                    pr = prp.tile([128, QW], F16, tag="pr")
                                    nc.vector.tensor_add(pr[:],
                                                         exs[step - 1][:],
                                                         ex[:])
                                    if ms == 1:
                                        acc = pr
                                    else:
                                        na = accp.tile([128, QW], F16,
                                                       tag="acc")
                                        nc.vector.tensor_add(na[:], acc[:],
                                                             pr[:])
                                        acc = na
                                    if ms == ST - 1:
                                        dacc_q[qc] = acc

                # ---------------- Phase 3: output projection (partial) ----------------
                with tc.tile_pool(name="ops", bufs=4, space="PSUM") as ops_pool, \
                     tc.tile_pool(name="ost", bufs=4) as ost:
                    for m in range(KT):
                        op = [ops_pool.tile([128, 1024], F32, tag="ops",
                                            name=f"op{m}_{qcc}")
                              for qcc in range(2)]
                        for j in range(HPG):
                            for qcc in range(2):
                                for ns in range(2):
                                    sl_o = slice(ns * 512, (ns + 1) * 512)
                                    nc.tensor.matmul(
                                        op[qcc][:, sl_o],
                                        w2s[m][:, j * 128:(j + 1) * 128],
                                        at_sb[:, j, qcc * 1024 + ns * 512:
                                              qcc * 1024 + (ns + 1) * 512],
                                        start=(j == 0), stop=(j == HPG - 1))
                        for qcc in range(2):
                            o16 = ost.tile([128, 1024], F16, tag="o16")
                            if qcc == 0:
                                nc.scalar.activation(o16[:], op[qcc][:], IDENT,
                                                     scale=1.0)
                            else:
                                nc.vector.tensor_copy(o16[:], op[qcc][:])
                            nc.sync.dma_start(
                                pout[m * 128:(m + 1) * 128,
                                     qcc * 1024:(qcc + 1) * 1024], o16[:])

    nc.compile()
    return nc


def _rope_tables():
    # Bug-faithful to the reference: exponent divides by EMB_DIM, not head_dim.
    angle = 1.0 / np.power(10000.0, np.arange(0, D, 2, dtype=np.float64) / E)
    t = np.arange(S, dtype=np.float64)
    freqs = np.repeat(t[:, None] * angle[None, :], 2, axis=-1)  # [S, D]
    return np.cos(freqs), np.sin(freqs)


def _prep_inputs(X, mask, W_qkv, b_qkv, W_o, b_o):
    """Build the 8 per-core input maps."""
    X = np.ascontiguousarray(np.asarray(X, dtype=np.float32))
    mask = np.asarray(mask)
    W_qkv = np.asarray(W_qkv, dtype=np.float32)
    b_qkv = np.asarray(b_qkv, dtype=np.float32)
    W_o = np.asarray(W_o, dtype=np.float32)

    cos, sin = _rope_tables()
    cosx = np.ascontiguousarray(cos.T.astype(np.float16))   # [D, S] fp16
    sinx = np.ascontiguousarray(sin.T.astype(np.float32))   # [D, S] f32
    ones = np.ones((128, 128), dtype=np.float16)
    # trans(q)[j] = -q[2j+1] (j<64), +q[2j-128] (j>=64), as lhsT: permT[d, j]
    permT = np.zeros((128, 128), dtype=np.float16)
    for j in range(64):
        permT[2 * j + 1, j] = -1.0
    for j in range(64, 128):
        permT[2 * (j - 64), j] = 1.0

    xts = [np.ascontiguousarray(X[b].T.astype(np.float16)) for b in range(B)]
    mbs = []
    for b in range(B):
        m = np.where(mask[b] == 0, np.float32(-1e9), np.float32(0.0)).astype(np.float32)
        mbs.append(np.ascontiguousarray(m.reshape(ST, 128).T))

    W1T = W_qkv.T                                           # [E, 3E]

    def pack_mtile(row0):
        # [128, E] with [p, k*128+c] = W1T[k*128+p, row0+c]
        blk = W1T[:, row0:row0 + 128].reshape(KT, 128, 128)  # [k, p, c]
        return blk.transpose(1, 0, 2).reshape(128, E).astype(np.float16)

    in_maps = []
    for c in range(NCORES):
        b, g = divmod(c, NGROUP)
        w1_rows = []
        bq_cols = []
        for hh in range(HPG):
            h = g * HPG + hh
            w1_rows.append(pack_mtile(h * D))               # Q_h
            bq_cols.append(b_qkv[h * D:(h + 1) * D])
            w1_rows.append(pack_mtile(E + h * D))           # K_h
            bq_cols.append(b_qkv[E + h * D:E + (h + 1) * D])
        w1p = np.concatenate(w1_rows, axis=0)               # [8*128, E]
        bqp = np.stack(bq_cols, axis=1).astype(np.float32)  # [128, 8]
        wv_rows = [pack_mtile(2 * E + (g * HPG + hh) * D) for hh in range(HPG)]
        wvp = np.concatenate(wv_rows, axis=0)               # [4*128, E]

        # w2: [m][p, j*128+c] = W_o^T[g*GE + j*128 + p, m*128+c]
        W2T = W_o.T[g * GE:(g + 1) * GE, :]                 # [512, E]
        w2p_ = W2T.reshape(HPG, 128, KT, 128).transpose(2, 1, 0, 3).reshape(
            KT * 128, GE).astype(np.float16)

        in_maps.append({
            "xt": xts[b],
            "w1": np.ascontiguousarray(w1p),
            "wv": np.ascontiguousarray(wvp),
            "w2": np.ascontiguousarray(w2p_),
            "bq": np.ascontiguousarray(bqp),
            "mb": mbs[b],
            "cosx": cosx,
            "sinx": sinx,
            "ones": ones,
            "perm": permT,
        })
    return in_maps


def kernel(X, mask, W_qkv, b_qkv, W_o, b_o, _trace=False):
    from concourse.bass_utils import run_bass_kernel_spmd

    if "nc" not in _CACHE:
        _CACHE["nc"] = _build()
    nc = _CACHE["nc"]

    in_maps = _prep_inputs(X, mask, W_qkv, b_qkv, W_o, b_o)
    res = run_bass_kernel_spmd(nc, in_maps, core_ids=list(range(NCORES)),
                               trace=_trace)
    _CACHE["last_result"] = res

    W_o = np.asarray(W_o, dtype=np.float32)
    b_o_eff = (np.asarray(b_o, dtype=np.float32)
               + W_o @ np.asarray(b_qkv, dtype=np.float32)[2 * E:])

    out = np.empty((B, S, E), dtype=np.float32)
    for b in range(B):
        acc = res.results[b * NGROUP]["pout"].astype(np.float32)
        for g in range(1, NGROUP):
            acc += res.results[b * NGROUP + g]["pout"].astype(np.float32)
        out[b] = acc.T + b_o_eff
    return out



# revision 6
# speedup vs baseline: 1.0862x; 1.0862x over previous
"""Trainium2 Bass kernel for a 16-head decoder self-attention block (v3, fp16).

Reference computation (B=2, S=2048, E=2048, H=16, D=128):
    qkv = X @ W_qkv.T + b_qkv ; RoPE(Q, K) ; attn = softmax(QK^T/sqrt(D) + mask)
    out = (attn @ V reshaped) @ W_o.T + b_o

Sharding over 8 NeuronCores: data parallel over batch (2) x tensor parallel
over 4 head-groups of 4 heads each. Each core computes its group's qkv
projection, attention, and a partial (rank-512) slice of the output
projection; the host sums the 4 partials per batch element.

v3 design vs v2:
  - V is projected directly in [s, d] layout (lhsT = X^T k-slice, moving =
    W_v^T) so the 256 per-head DMA XBAR transposes (80us of queue time and a
    7us PE stall at the phase boundary) disappear entirely.
  - One flat software-pipelined attention stream over (qc, head, ms): the
    exp/AV/denominator machinery crosses group boundaries without draining,
    so the PE never waits for a head transition.
  - The serialized ACT exp stream (1005ns per [128,1024] tile, the phase-2
    floor) gets a head start: ~11 leading score/exp steps are pre-rolled
    into the V-projection phase, which has no ACT work of its own.
  - PSUM is split 4/3/1: scores ring 2x[128,1024], AV accumulators as
    [128,512] halves ring 3 (also reused for the RoPE perm and V psums),
    and a 1-bank denominator slot.
  - Output projection emits immediately after the stream with no barrier.
"""

import contextlib
import math
import sys

import numpy as np

sys.path.insert(0, "/opt/trn_rl_repo")

B, S, E = 2, 2048, 2048
H, D = 16, 128
NCORES = 8
NGROUP = 4          # head groups (tensor parallel)
HPG = H // NGROUP   # heads per group = 4
GE = HPG * D        # group embed width = 512
KT = E // 128       # contraction tiles over E = 16
ST = S // 128       # sequence tiles = 16
SCALE = 1.0 / math.sqrt(D)
NQC = 2             # query chunks of 1024
QW = S // NQC
NGRP = NQC * HPG    # 8 attention (qc, head) groups per core
NSTEP = NGRP * ST   # 128 leading/trailing steps
TRAIL = 3
PREROLL = 11

_CACHE = {}


def _build():
    """Build + compile the per-core Bass program (same program, all cores)."""
    import concourse.bacc as bacc
    import concourse.mybir as mybir
    import concourse.tile as tile

    F32 = mybir.dt.float32
    F16 = mybir.dt.float16
    EXP = mybir.ActivationFunctionType.Exp
    IDENT = mybir.ActivationFunctionType.Identity

    nc = bacc.Bacc("TRN2", target_bir_lowering=False, debug=False)

    xt = nc.dram_tensor("xt", [E, S], F16, kind="ExternalInput").ap()        # X[b].T
    w1 = nc.dram_tensor("w1", [8 * 128, E], F16, kind="ExternalInput").ap()  # Q/K m-tiles
    wvt = nc.dram_tensor("wvt", [128, KT * GE], F16, kind="ExternalInput").ap()
    w2 = nc.dram_tensor("w2", [KT * 128, GE], F16, kind="ExternalInput").ap()
    bq = nc.dram_tensor("bq", [128, 8], F32, kind="ExternalInput").ap()
    mb = nc.dram_tensor("mb", [128, ST], F32, kind="ExternalInput").ap()
    cosx = nc.dram_tensor("cosx", [128, S], F16, kind="ExternalInput").ap()
    sinx = nc.dram_tensor("sinx", [128, S], F16, kind="ExternalInput").ap()
    ones = nc.dram_tensor("ones", [128, 128], F16, kind="ExternalInput").ap()
    perm = nc.dram_tensor("perm", [128, 128], F16, kind="ExternalInput").ap()
    pout = nc.dram_tensor("pout", [E, S], F16, kind="ExternalOutput").ap()

    with tile.TileContext(nc) as tc, contextlib.ExitStack() as est:
            spool = est.enter_context(tc.tile_pool(name="small", bufs=1))
            qkpool = est.enter_context(tc.tile_pool(name="qk", bufs=1))
            vpool = est.enter_context(tc.tile_pool(name="vsb", bufs=1))
            expool = est.enter_context(tc.tile_pool(name="ex", bufs=15))
            prp = est.enter_context(tc.tile_pool(name="pr", bufs=3))
            accp = est.enter_context(tc.tile_pool(name="acc", bufs=4))
            rcp = est.enter_context(tc.tile_pool(name="rc", bufs=2))
            pss_pool = est.enter_context(tc.tile_pool(name="pss", bufs=2, space="PSUM"))
            psoh_pool = est.enter_context(tc.tile_pool(name="psoh", bufs=3, space="PSUM"))
            psd_pool = est.enter_context(tc.tile_pool(name="psd", bufs=1, space="PSUM"))
            ones_sb = spool.tile([128, 128], F16, tag="ones")
            mb_sb = spool.tile([128, ST], F32, tag="mb")
            bq_sb = spool.tile([128, 8], F32, tag="bq")
            perm_sb = spool.tile([128, 128], F16, tag="perm")

            qk_sb = [qkpool.tile([128, S], F16, tag=f"qk{m}", name=f"qk{m}")
                     for m in range(8)]
            v_sb = vpool.tile([128, ST, GE], F16, tag="vsb", name="v_sb")

            # ---- attention stream machinery (closures over shared state) ----
            exs = {}     # leading step -> exp tile
            chain = {}   # group -> running denominator accumulator
            dacc = {}    # group -> final denominator (pre cross-partition)
            rcs = {}     # (group, ns) -> reciprocal tile
            psos = {}    # group -> [pso half ns=0, ns=1]
            at_ref = []  # filled with at_sb once allocated

            def emit_lead(gl):
                grp, ms = gl // ST, gl % ST
                qc, h = grp // HPG, grp % HPG
                qt, kt_ = qk_sb[2 * h], qk_sb[2 * h + 1]
                ps = pss_pool.tile([128, QW], F32, tag="pss")
                for ns in range(2):
                    nc.tensor.matmul(
                        ps[:, ns * 512:(ns + 1) * 512],
                        kt_[:, ms * 128:(ms + 1) * 128],
                        qt[:, qc * QW + ns * 512: qc * QW + (ns + 1) * 512],
                        start=True, stop=True)
                ex = expool.tile([128, QW], F16, tag="ex")
                nc.scalar.activation(ex[:], ps[:], EXP,
                                     bias=mb_sb[:, ms:ms + 1], scale=SCALE)
                exs[gl] = ex
                if ms % 2 == 1:
                    pr = prp.tile([128, QW], F16, tag="pr")
                    nc.vector.tensor_add(pr[:], exs[gl - 1][:], ex[:])
                    if ms == 1:
                        chain[grp] = pr
                    else:
                        na = accp.tile([128, QW], F16, tag="acc")
                        nc.vector.tensor_add(na[:], chain[grp][:], pr[:])
                        chain[grp] = na
                    if ms == ST - 1:
                        dacc[grp] = chain[grp]

            def emit_trail(gt):
                grp, pms = gt // ST, gt % ST
                qc, h = grp // HPG, grp % HPG
                if pms == 0:
                    psos[grp] = [psoh_pool.tile([128, 512], F32, tag="pso",
                                                name=f"pso{grp}_{ns}")
                                 for ns in range(2)]
                po = psos[grp]
                ex = exs[gt]
                for ns in range(2):
                    nc.tensor.matmul(po[ns][:],
                                     v_sb[:, pms, h * 128:(h + 1) * 128],
                                     ex[:, ns * 512:(ns + 1) * 512],
                                     start=(pms == 0), stop=(pms == ST - 1))
                if pms in (ST - 3, ST - 2):
                    ns = pms - (ST - 3)
                    psd = psd_pool.tile([128, 512], F32, tag="psd")
                    nc.tensor.matmul(psd[:], ones_sb[:],
                                     dacc[grp][:, ns * 512:(ns + 1) * 512],
                                     start=True, stop=True)
                    rc = rcp.tile([128, 512], F32, tag="rc")
                    nc.vector.reciprocal_approx_fast(rc[:], psd[:])
                    rcs[(grp, ns)] = rc
                if pms == ST - 1:
                    at_sb = at_ref[0]
                    for ns in range(2):
                        qsl = slice(qc * QW + ns * 512, qc * QW + (ns + 1) * 512)
                        nc.vector.tensor_mul(at_sb[:, h, qsl], po[ns][:],
                                             rcs[(grp, ns)][:])

            lead_i = 0

            # ================= Phase 1: projections =================
            with contextlib.ExitStack() as est1:
                xpool = est1.enter_context(tc.tile_pool(name="xt", bufs=1))
                wvtp = est1.enter_context(tc.tile_pool(name="wvt", bufs=1))
                w1p = est1.enter_context(tc.tile_pool(name="w1p", bufs=2))
                trig = est1.enter_context(tc.tile_pool(name="trig", bufs=1))
                qbp = est1.enter_context(tc.tile_pool(name="qbp", bufs=2))
                rap = est1.enter_context(tc.tile_pool(name="rap", bufs=2))
                stp = est1.enter_context(tc.tile_pool(name="stp", bufs=2))
                # DMA order: first m-tile weights, then X first halves (the
                # m0/half0 k-loop chases this stream), then the rest.
                w1t = [None] * 8
                w1t[0] = w1p.tile([128, E], F16, tag="w", name="w1_0")
                nc.sync.dma_start(w1t[0][:], w1[0:128, :])
                xts = []
                for k in range(KT):
                    t = xpool.tile([128, S], F16, tag=f"xt{k}", name=f"xts{k}")
                    nc.sync.dma_start(t[:, 0:1024],
                                      xt[k * 128:(k + 1) * 128, 0:1024])
                    xts.append(t)
                nc.sync.dma_start(bq_sb[:], bq)
                nc.sync.dma_start(perm_sb[:], perm)
                cos_sb = trig.tile([128, S], F16, tag="cos")
                nc.sync.dma_start(cos_sb[:], cosx)
                for k in range(KT):
                    nc.sync.dma_start(xts[k][:, 1024:2048],
                                      xt[k * 128:(k + 1) * 128, 1024:2048])
                sin_sb = trig.tile([128, S], F16, tag="sin")
                nc.sync.dma_start(sin_sb[:], sinx)
                wvt_sb = wvtp.tile([128, KT * GE], F16, tag="wvt")
                for c in range(4):
                    cs = slice(c * 4 * GE, (c + 1) * 4 * GE)
                    nc.sync.dma_start(wvt_sb[:, cs], wvt[:, cs])
                nc.sync.dma_start(mb_sb[:], mb)
                nc.sync.dma_start(ones_sb[:], ones)

                # ---- Phase 1a: Q/K projections + RoPE ----
                for m in range(8):
                    if m + 1 < 8:
                        w1t[m + 1] = w1p.tile([128, E], F16, tag="w",
                                              name=f"w1_{m + 1}")
                        nc.sync.dma_start(w1t[m + 1][:],
                                          w1[(m + 1) * 128:(m + 2) * 128, :])
                    wt = w1t[m]
                    for half in range(2):
                        hs = slice(half * 1024, (half + 1) * 1024)
                        ps = pss_pool.tile([128, 1024], F32, tag="pss")
                        for k in range(KT):
                            for ns in range(2):
                                nc.tensor.matmul(
                                    ps[:, ns * 512:(ns + 1) * 512],
                                    wt[:, k * 128:(k + 1) * 128],
                                    xts[k][:, half * 1024 + ns * 512:
                                            half * 1024 + (ns + 1) * 512],
                                    start=(k == 0), stop=(k == KT - 1))
                        qb = qbp.tile([128, 1024], F16, tag="qb")
                        nc.scalar.activation(qb[:], ps[:], IDENT,
                                             bias=bq_sb[:, m:m + 1], scale=1.0)
                        p2 = []
                        for ns in range(2):
                            t = psoh_pool.tile([128, 512], F32, tag="pso")
                            nc.tensor.matmul(t[:], perm_sb[:],
                                             qb[:, ns * 512:(ns + 1) * 512],
                                             start=True, stop=True)
                            p2.append(t)
                        ra = rap.tile([128, 1024], F16, tag="ra")
                        nc.vector.tensor_mul(ra[:], qb[:], cos_sb[:, hs])
                        st = stp.tile([128, 1024], F16, tag="st")
                        for ns in range(2):
                            ssl = slice(half * 1024 + ns * 512,
                                        half * 1024 + (ns + 1) * 512)
                            nc.vector.tensor_mul(st[:, ns * 512:(ns + 1) * 512],
                                                 p2[ns][:], sin_sb[:, ssl])
                        nc.vector.tensor_add(qk_sb[m][:, hs], st[:], ra[:])

                # ---- Phase 1b: V projection ([s, d] layout) + pre-rolled
                # leading attention steps (scores+exp have no V dependency) ----
                for st_ in range(ST):
                    vp = psoh_pool.tile([128, 512], F32, tag="pso")
                    for k in range(KT):
                        nc.tensor.matmul(vp[:],
                                         xts[k][:, st_ * 128:(st_ + 1) * 128],
                                         wvt_sb[:, k * GE:(k + 1) * GE],
                                         start=(k == 0), stop=(k == KT - 1))
                    nc.vector.tensor_copy(v_sb[:, st_, :], vp[:])
                    if st_ >= 4 and lead_i < PREROLL:
                        emit_lead(lead_i)
                        lead_i += 1

            # xts / wvt / w1 / trig freed here; at_sb + w2 take their place
            with contextlib.ExitStack() as est2:
                atp = est2.enter_context(tc.tile_pool(name="at", bufs=1))
                w2p = est2.enter_context(tc.tile_pool(name="w2p", bufs=1))
                ost = est2.enter_context(tc.tile_pool(name="ost", bufs=4))
                at_sb = atp.tile([128, HPG, S], F16, tag="at", name="at_sb")
                at_ref.append(at_sb)
                w2s = []
                for m in range(KT):
                    t = w2p.tile([128, GE], F16, tag=f"w2_{m}", name=f"w2_{m}")
                    nc.sync.dma_start(t[:], w2[m * 128:(m + 1) * 128, :])
                    w2s.append(t)

                # ---- Phase 2: flat attention stream ----
                trail_i = 0
                while trail_i < NSTEP:
                    if lead_i < NSTEP:
                        emit_lead(lead_i)
                        lead_i += 1
                    budget = 2 if (lead_i - trail_i) > TRAIL + 1 else 1
                    if lead_i >= NSTEP:
                        budget = NSTEP - trail_i
                    for _ in range(budget):
                        if trail_i < NSTEP and trail_i < lead_i:
                            emit_trail(trail_i)
                            trail_i += 1

                # ---- Phase 3: output projection (partial) ----
                for qcc in range(2):
                    for m in range(KT):
                        op = pss_pool.tile([128, 1024], F32, tag="pss")
                        for j in range(HPG):
                            for ns in range(2):
                                nc.tensor.matmul(
                                    op[:, ns * 512:(ns + 1) * 512],
                                    w2s[m][:, j * 128:(j + 1) * 128],
                                    at_sb[:, j, qcc * 1024 + ns * 512:
                                          qcc * 1024 + (ns + 1) * 512],
                                    start=(j == 0), stop=(j == HPG - 1))
                        o16 = ost.tile([128, 1024], F16, tag="o16")
                        if (qcc * KT + m) % 2 == 0:
                            nc.scalar.activation(o16[:], op[:], IDENT, scale=1.0)
                        else:
                            nc.vector.tensor_copy(o16[:], op[:])
                        nc.sync.dma_start(
                            pout[m * 128:(m + 1) * 128,
                                 qcc * 1024:(qcc + 1) * 1024], o16[:])

    nc.compile()
    return nc


def _rope_tables():
    # Bug-faithful to the reference: exponent divides by EMB_DIM, not head_dim.
    angle = 1.0 / np.power(10000.0, np.arange(0, D, 2, dtype=np.float64) / E)
    t = np.arange(S, dtype=np.float64)
    freqs = np.repeat(t[:, None] * angle[None, :], 2, axis=-1)  # [S, D]
    return np.cos(freqs), np.sin(freqs)


def _prep_inputs(X, mask, W_qkv, b_qkv, W_o, b_o):
    """Build the 8 per-core input maps."""
    X = np.ascontiguousarray(np.asarray(X, dtype=np.float32))
    mask = np.asarray(mask)
    W_qkv = np.asarray(W_qkv, dtype=np.float32)
    b_qkv = np.asarray(b_qkv, dtype=np.float32)
    W_o = np.asarray(W_o, dtype=np.float32)

    cos, sin = _rope_tables()
    cosx = np.ascontiguousarray(cos.T.astype(np.float16))   # [D, S] fp16
    sinx = np.ascontiguousarray(sin.T.astype(np.float16))   # [D, S] fp16
    ones = np.ones((128, 128), dtype=np.float16)
    # trans(q)[j] = -q[2j+1] (j<64), +q[2j-128] (j>=64), as lhsT: permT[d, j]
    permT = np.zeros((128, 128), dtype=np.float16)
    for j in range(64):
        permT[2 * j + 1, j] = -1.0
    for j in range(64, 128):
        permT[2 * (j - 64), j] = 1.0

    xts = [np.ascontiguousarray(X[b].T.astype(np.float16)) for b in range(B)]
    mbs = []
    for b in range(B):
        m = np.where(mask[b] == 0, np.float32(-1e9), np.float32(0.0)).astype(np.float32)
        mbs.append(np.ascontiguousarray(m.reshape(ST, 128).T))

    W1T = W_qkv.T                                           # [E, 3E]

    def pack_mtile(row0):
        # [128, E] with [p, k*128+c] = W1T[k*128+p, row0+c]
        blk = W1T[:, row0:row0 + 128].reshape(KT, 128, 128)  # [k, p, c]
        return blk.transpose(1, 0, 2).reshape(128, E).astype(np.float16)

    in_maps = []
    for c in range(NCORES):
        b, g = divmod(c, NGROUP)
        w1_rows = []
        bq_cols = []
        for hh in range(HPG):
            h = g * HPG + hh
            w1_rows.append(pack_mtile(h * D))               # Q_h
            bq_cols.append(b_qkv[h * D:(h + 1) * D])
            w1_rows.append(pack_mtile(E + h * D))           # K_h
            bq_cols.append(b_qkv[E + h * D:E + (h + 1) * D])
        w1p = np.concatenate(w1_rows, axis=0)               # [8*128, E]
        bqp = np.stack(bq_cols, axis=1).astype(np.float32)  # [128, 8]

        # wvt: [p, k*GE + j] = W_v_group[j, k*128+p]
        Wv_g = W_qkv[2 * E + g * GE: 2 * E + (g + 1) * GE, :]   # [512, E]
        wvt_ = Wv_g.reshape(GE, KT, 128).transpose(2, 1, 0).reshape(
            128, KT * GE).astype(np.float16)

        # w2: [m][p, j*128+c] = W_o^T[g*GE + j*128 + p, m*128+c]
        W2T = W_o.T[g * GE:(g + 1) * GE, :]                 # [512, E]
        w2p_ = W2T.reshape(HPG, 128, KT, 128).transpose(2, 1, 0, 3).reshape(
            KT * 128, GE).astype(np.float16)

        in_maps.append({
            "xt": xts[b],
            "w1": np.ascontiguousarray(w1p),
            "wvt": np.ascontiguousarray(wvt_),
            "w2": np.ascontiguousarray(w2p_),
            "bq": np.ascontiguousarray(bqp),
            "mb": mbs[b],
            "cosx": cosx,
            "sinx": sinx,
            "ones": ones,
            "perm": permT,
        })
    return in_maps


def kernel(X, mask, W_qkv, b_qkv, W_o, b_o, _trace=False):
    from concourse.bass_utils import run_bass_kernel_spmd

    if "nc" not in _CACHE:
        _CACHE["nc"] = _build()
    nc = _CACHE["nc"]

    in_maps = _prep_inputs(X, mask, W_qkv, b_qkv, W_o, b_o)
    res = run_bass_kernel_spmd(nc, in_maps, core_ids=list(range(NCORES)),
                               trace=_trace)
    _CACHE["last_result"] = res

    W_o = np.asarray(W_o, dtype=np.float32)
    b_o_eff = (np.asarray(b_o, dtype=np.float32)
               + W_o @ np.asarray(b_qkv, dtype=np.float32)[2 * E:])

    out = np.empty((B, S, E), dtype=np.float32)
    for b in range(B):
        acc = res.results[b * NGROUP]["pout"].astype(np.float32)
        for g in range(1, NGROUP):
            acc += res.results[b * NGROUP + g]["pout"].astype(np.float32)
        out[b] = acc.T + b_o_eff
    return out
